# revision 67
# baseline (speedup 1.0000x reference)
"""Multi-head causal self-attention (B=2, S=2048, D=1024, H=16) on 8 TRN2 cores.

Sharding: core = b*4 + hg  (b in {0,1} batch, hg in {0..3} head-group of 4 heads).
Per core: project qT/kT (pair-packed [128, S], bf16) and v ([S, 64] blocks, bf16),
compute transposed scores S^T = K Q^T per head (k on partitions), causal mask
added in PSUM via identity-matmul, exp on ScalarE (bf16 out), PV matmul with a
ones-column appended to V so row 64 of the accumulator is the softmax sum,
normalization via reciprocal + DMA partition-broadcast + tensor mul, then the
partial output projection. Host sums the 4 per-batch partials and adds
(b_v @ w_o.T + b_o); b_k is dropped (softmax is invariant to per-query
constants); b_q is applied on-device. Matmul operands are bf16 (fp32 moving
operand streams at half rate on TRN2); all accumulation is fp32 in PSUM.
"""

import numpy as np
import ml_dtypes

import concourse.bass as bass
import concourse.mybir as mybir
import concourse.tile as tile
from concourse import bacc
from concourse.bass_utils import run_bass_kernel_spmd

B, S, D, H, DK = 2, 2048, 1024, 16, 64
N_CORES = 8
F32 = mybir.dt.float32
BF16 = mybir.dt.bfloat16
NPBF = ml_dtypes.bfloat16
AF = mybir.ActivationFunctionType
NEG_BIG = -1.0e9


def _build(debug=False):
    nc = bacc.Bacc("TRN2", target_bir_lowering=False, debug=False,
                   num_devices=N_CORES)
    xT = nc.dram_tensor("xT", [D, S], BF16, kind="ExternalInput").ap()
    wqT = nc.dram_tensor("wqT", [D, 256], BF16, kind="ExternalInput").ap()
    wkT = nc.dram_tensor("wkT", [D, 256], BF16, kind="ExternalInput").ap()
    wvT = nc.dram_tensor("wvT", [D, 256], BF16, kind="ExternalInput").ap()
    woT = nc.dram_tensor("woT", [256, D], BF16, kind="ExternalInput").ap()
    bq2 = nc.dram_tensor("bq2", [128, 2], F32, kind="ExternalInput").ap()
    tri = nc.dram_tensor("tri", [128, 128], BF16, kind="ExternalInput").ap()
    y = nc.dram_tensor("y", [S, D], BF16, kind="ExternalOutput").ap()
    dbg = {}
    if debug:
        for nm, shp in [("qT", [128, 2, S]), ("kT", [128, 2, S]),
                        ("vv", [128, 16, 260]), ("oT", [128, 2, S])]:
            dbg[nm] = nc.dram_tensor(nm, shp, BF16, kind="ExternalOutput").ap()

    NQC = 4          # q-chunks of 512
    QC = 512
    NKT = S // 128   # k tiles

    with tile.TileContext(nc) as tc, \
            nc.allow_low_precision(reason="bf16 attention kernel"):
        with (
            tc.tile_pool(name="persist", bufs=1) as persist,
            tc.tile_pool(name="kqv", bufs=2) as kqv,
        ):
            qT_sb = [kqv.tile([128, S], BF16, tag="qT", name=f"qT{p}") for p in range(2)]
            kT_sb = [kqv.tile([128, S], BF16, tag="kT", name=f"kT{p}") for p in range(2)]
            v_sb = [persist.tile([128, 4 * 65], BF16, tag=f"v{t}", name=f"v{t}")
                    for t in range(NKT)]
            outT_sb = [persist.tile([128, S], BF16, tag=f"oT{p}", name=f"oTs{p}")
                       for p in range(2)]
            wo_sb = [persist.tile([128, D], BF16, tag=f"wo{p}", name=f"wo{p}")
                     for p in range(2)]
            tri_sb = persist.tile([128, 128], BF16, tag="tri")
            bq_sb = persist.tile([128, 2], F32, tag="bq")

            with (
                tc.tile_pool(name="xw", bufs=1) as xw,
                tc.tile_pool(name="ep", bufs=7) as ep,
                tc.tile_pool(name="rp", bufs=6) as rp,
            ):
                xt = [xw.tile([128, S], BF16, tag=f"x{c}", name=f"xt{c}") for c in range(8)]
                wq_sb = [xw.tile([128, 256], BF16, tag=f"wq{c}", name=f"wqs{c}") for c in range(8)]
                wk_sb = [xw.tile([128, 256], BF16, tag=f"wk{c}", name=f"wks{c}") for c in range(8)]
                wv_sb = [xw.tile([128, 256], BF16, tag=f"wv{c}", name=f"wvs{c}") for c in range(8)]
                # xt on the sync HWDGE queue; wq on the scalar HWDGE queue;
                # wk/wv interleaved on gpsimd so chunk c lands just before
                # the xt chunk that gates its preamble matmul.
                for c in range(8):
                    nc.sync.dma_start(out=xt[c], in_=xT[c * 128:(c + 1) * 128, :])
                for c in range(8):
                    nc.scalar.dma_start(out=wq_sb[c], in_=wqT[c * 128:(c + 1) * 128, :])
                for c in range(8):
                    nc.gpsimd.dma_start(out=wk_sb[c], in_=wkT[c * 128:(c + 1) * 128, :])
                    nc.gpsimd.dma_start(out=wv_sb[c], in_=wvT[c * 128:(c + 1) * 128, :])
                nc.gpsimd.dma_start(out=bq_sb, in_=bq2)
                nc.gpsimd.dma_start(out=tri_sb, in_=tri)
                for p in range(2):
                    nc.gpsimd.dma_start(out=wo_sb[p], in_=woT[p * 128:(p + 1) * 128, :])

                # trace-order schedule validator: a read before its write in
                # trace order silently reads garbage, so assert every block's
                # needs were emitted earlier.
                written = set()

                def qk_chain(p, j, which, pool):
                    written.add((which, p, j))
                    ps = pool.tile([128, QC], F32, tag="proj", name="ps")
                    w_sb = wq_sb if which == "q" else wk_sb
                    for c in range(8):
                        nc.tensor.matmul(
                            ps, w_sb[c][:, p * 128:(p + 1) * 128],
                            xt[c][:, j * QC:(j + 1) * QC],
                            start=(c == 0), stop=(c == 7))
                    if which == "q":
                        nc.vector.tensor_scalar_add(
                            qT_sb[p][:, j * QC:(j + 1) * QC], ps, bq_sb[:, p:p + 1])
                    else:
                        nc.vector.tensor_copy(kT_sb[p][:, j * QC:(j + 1) * QC], ps)

                def v_chain(t, pool):
                    written.add(("v", t))
                    ps_v = pool.tile([128, 256], F32, tag="proj", name="ps_v")
                    for c in range(8):
                        nc.tensor.matmul(
                            ps_v, xt[c][:, t * 128:(t + 1) * 128], wv_sb[c],
                            start=(c == 0), stop=(c == 7))
                    v_view = v_sb[t].rearrange("p (h w) -> p h w", w=65)
                    nc.vector.memset(v_view[:, :, 64:65], 1.0)
                    nc.vector.tensor_copy(
                        v_view[:, :, 0:64],
                        ps_v.rearrange("p (h w) -> p h w", w=64))

                def emit_norm(p_, q0_, o_ps_, tail=False):
                    written.add(("outT", p_, q0_ // QC))
                    # recip of the ones-row sum (copied to SBUF first —
                    # reciprocal_approx_fast from PSUM returns garbage),
                    # broadcast along partitions on GpSimd, fused multiply
                    # from PSUM into the bf16 outT tile. In the kernel tail
                    # (last block) the sums copy goes to the idle ScalarE and
                    # the multiply is split per q-tile so the output
                    # projection units start sooner.
                    if tail:
                        # 256-wide chunks: the first output-projection units
                        # unlock after one chunk-chain (~2us) instead of the
                        # full-width chain (~4us), keeping the PE-idle gap
                        # under the HAM re-throttle window.
                        for qq in range(2):
                            cs = slice(qq * 256, (qq + 1) * 256)
                            for s in range(2):
                                sums = rp.tile([1, 256], F32, tag="sums_t",
                                               name="sums_t")
                                nc.scalar.activation(
                                    sums, o_ps_[s][64:65, cs], AF.Copy)
                                recip = rp.tile([1, 256], F32, tag="recip_t",
                                                name="recip_t")
                                nc.vector.reciprocal_approx_fast(
                                    out=recip, in_=sums)
                                bc = rp.tile([64, 256], F32, tag="bc_t",
                                             name="bc_t")
                                nc.gpsimd.partition_broadcast(bc, recip)
                                nc.vector.tensor_mul(
                                    outT_sb[p_][s * 64:(s + 1) * 64,
                                                q0_ + qq * 256:q0_ + (qq + 1) * 256],
                                    o_ps_[s][0:64, cs], bc)
                        return
                    for s in range(2):
                        sums = rp.tile([1, QC], F32, tag="sums", name="sums")
                        nc.scalar.activation(sums, o_ps_[s][64:65, :], AF.Copy)
                        recip = rp.tile([1, QC], F32, tag="recip", name="recip")
                        nc.vector.reciprocal_approx_fast(out=recip, in_=sums)
                        bc = rp.tile([64, QC], F32, tag="bc", name="bc")
                        nc.gpsimd.partition_broadcast(bc, recip)
                        nc.vector.tensor_mul(
                            outT_sb[p_][s * 64:(s + 1) * 64, q0_:q0_ + QC],
                            o_ps_[s][0:64, :], bc)

                def emit_pv(p, o_ps, nkt, ent):
                    _kt, _e, _lo = ent
                    for s in range(2):
                        hb = 2 * p + s
                        nc.tensor.matmul(
                            o_ps[s][:, _lo:QC],
                            v_sb[_kt][:, hb * 65:(hb + 1) * 65],
                            _e[:, s * QC + _lo:(s + 1) * QC],
                            start=(_kt == 0), stop=(_kt == nkt - 1),
                            skip_group_check=True)

                def emit_block(p, qc, pops, tail=False):
                    assert ("q", p, qc) in written, (p, qc, "q")
                    for j in range(qc + 1):
                        assert ("k", p, j) in written, (p, qc, "k", j)
                    for t in range(4 * qc + 4):
                        assert ("v", t) in written, (p, qc, "v", t)
                    q0 = qc * QC
                    nkt = 4 * qc + 4
                    o_ps = [opp.tile([65, QC], F32, tag=f"o{s}", name=f"ops{s}")
                            for s in range(2)]
                    pend = []
                    for kt in range(nkt):
                        o = kt * 128 - q0
                        diag = o >= 0
                        lo = o if diag else 0
                        s_ab = sqp.tile([128, 2 * QC], F32, tag="sq", name="s_ab")
                        for s in range(2):
                            half = s * QC
                            nc.tensor.matmul(
                                s_ab[:, half + lo:half + QC],
                                kT_sb[p][s * 64:(s + 1) * 64,
                                         kt * 128:(kt + 1) * 128],
                                qT_sb[p][s * 64:(s + 1) * 64,
                                         q0 + lo:q0 + QC],
                                start=True, stop=True,
                                tile_position=(s * 64, 0),
                                skip_group_check=True)
                        e_ab = ep.tile([128, 2 * QC], BF16, tag="e", name="e_ab")
                        if diag:
                            # one ACTIVATE spanning both heads; cols
                            # [QC:QC+lo] hold exp(stale psum) and are
                            # never read downstream.
                            nc.scalar.activation(
                                e_ab[:, lo:2 * QC], s_ab[:, lo:2 * QC],
                                AF.Exp, scale=0.125)
                            for s in range(2):
                                nc.vector.tensor_mul(
                                    e_ab[:, s * QC + o:s * QC + o + 128],
                                    e_ab[:, s * QC + o:s * QC + o + 128],
                                    tri_sb)
                        else:
                            nc.scalar.activation(e_ab, s_ab, AF.Exp, scale=0.125)
                        # PV lags scores by 3 kt: exp(kt) gets ~3 PE rounds of
                        # slack, the first PV of a block lands after the prior
                        # block's norm has released the o_ps slots, and the
                        # end-of-block flush leaves a PE burst at the boundary.
                        if len(pend) == 4:
                            emit_pv(p, o_ps, nkt, pend.pop(0))
                        pops(kt, nkt)
                        pend.append((kt, e_ab, lo))
                    while pend:
                        emit_pv(p, o_ps, nkt, pend.pop(0))
                    emit_norm(p, q0, o_ps, tail=tail)

                # ---- preamble: pair-0 j0 q/k + v0-3, interleaved c-major so
                # every arriving x chunk unlocks 6 matmuls. (Extending this
                # with pair-1 j0 measured 200us vs 164.5 — do not retry.) ----
                with tc.tile_pool(name="ppsA", bufs=6, space="PSUM") as ppsA:
                    ps_q0 = ppsA.tile([128, QC], F32, tag="projA", name="ps_q0")
                    ps_k0 = ppsA.tile([128, QC], F32, tag="projA", name="ps_k0")
                    ps_vh = [ppsA.tile([128, 256], F32, tag="projA",
                                       name=f"ps_vh{t}") for t in range(4)]
                    for c in range(8):
                        nc.tensor.matmul(
                            ps_q0, wq_sb[c][:, 0:128], xt[c][:, 0:QC],
                            start=(c == 0), stop=(c == 7))
                        nc.tensor.matmul(
                            ps_k0, wk_sb[c][:, 0:128], xt[c][:, 0:QC],
                            start=(c == 0), stop=(c == 7))
                        for t in range(4):
                            nc.tensor.matmul(
                                ps_vh[t], xt[c][:, t * 128:(t + 1) * 128],
                                wv_sb[c], start=(c == 0), stop=(c == 7))
                    written.update({("q", 0, 0), ("k", 0, 0),
                                    ("v", 0), ("v", 1), ("v", 2), ("v", 3)})
                    # kT copy on ScalarE (idle here), first k-tile's columns
                    # first, in parallel with the DVE qT bias-add — unblocks
                    # the first scores matmuls ~1us earlier.
                    nc.scalar.activation(
                        kT_sb[0][:, 0:128], ps_k0[:, 0:128], AF.Copy)
                    nc.vector.tensor_scalar_add(
                        qT_sb[0][:, 0:QC], ps_q0, bq_sb[:, 0:1])
                    nc.scalar.activation(
                        kT_sb[0][:, 128:QC], ps_k0[:, 128:QC], AF.Copy)
                    for t in range(4):
                        v_view = v_sb[t].rearrange("p (h w) -> p h w", w=65)
                        nc.vector.memset(v_view[:, :, 64:65], 1.0)
                        nc.vector.tensor_copy(
                            v_view[:, :, 0:64],
                            ps_vh[t].rearrange("p (h w) -> p h w", w=64))
                # ---- interleaved pair-0 / pair-1 attention blocks, with
                # projection chains and output-projection units as fillers ----
                with (
                    tc.tile_pool(name="sq", bufs=2, space="PSUM") as sqp,
                    tc.tile_pool(name="ops", bufs=1, space="PSUM") as opp,
                    tc.tile_pool(name="aux", bufs=2, space="PSUM") as aux,
                    tc.tile_pool(name="fsb", bufs=4) as fsb,
                ):
                    f_hold = {}

                    def c_unit(qt, oc):
                        for p_ in range(2):
                            assert ("outT", p_, qt // 4) in written, (qt, oc, p_)
                        f_ps = aux.tile([128, 512], F32, tag="proj", name="f_ps")
                        for p in range(2):
                            nc.tensor.matmul(
                                f_ps, outT_sb[p][:, qt * 128:(qt + 1) * 128],
                                wo_sb[p][:, oc * 512:(oc + 1) * 512],
                                start=(p == 0), stop=(p == 1))
                        if oc == 0:
                            f_sb = fsb.tile([128, 1024], BF16, tag="f",
                                            name="f_sb")
                            f_hold[qt] = f_sb
                        else:
                            f_sb = f_hold.pop(qt)
                        nc.vector.tensor_copy(
                            f_sb[:, oc * 512:(oc + 1) * 512], f_ps)
                        if oc == 1:
                            nc.sync.dma_start(
                                out=y[qt * 128:(qt + 1) * 128, :], in_=f_sb)

                    def qk(p, j, w):
                        return lambda: qk_chain(p, j, w, aux)

                    def vch(t):
                        return lambda: v_chain(t, aux)

                    # Block order mixes ScalarE-heavy attention with PE-heavy
                    # projections and ends on the smallest block (1,1) to
                    # shrink the tail.
                    blocks = [(0, 0), (0, 1), (1, 0), (0, 2),
                              (1, 1), (0, 3), (1, 2), (1, 3)]
                    # deadline table: block (p,qc) needs its pair's k chunks
                    # j<=qc and q chunk j=qc written in an EARLIER block (the
                    # preamble covers pair-0 j0 and v0-3).
                    # fillers pop as late as their deadline allows so the
                    # ScalarE-paced late blocks keep the PE warm.
                    queues = {i: [] for i in range(8)}
                    queues[0] = [qk(0, 1, "q"), qk(0, 1, "k"),
                                 vch(4), vch(5), vch(6), vch(7)]
                    queues[1] = [qk(1, 0, "k"), qk(1, 0, "q"),
                                 qk(0, 2, "q"), qk(0, 2, "k"),
                                 vch(8), vch(9), vch(10), vch(11)]
                    queues[2] = [qk(1, 1, "k"), qk(1, 1, "q"), vch(12), vch(13)]
                    queues[4] = [qk(0, 3, "q"), qk(0, 3, "k"), vch(14), vch(15),
                                 qk(1, 2, "k"), qk(1, 2, "q")]
                    queues[5] = [qk(1, 3, "k"), qk(1, 3, "q")]
                    # c_unit routing: units for qc_j may only run after BOTH
                    # (0,j) and (1,j) blocks have produced outT for qt range.
                    unit_route = {2: [(3, 8)], 4: [(5, 4), (6, 4)], 6: [(7, 8)]}

                    # queues 3 and 7 hold only c_units produced by the
                    # immediately-preceding block; popping them at kt=0 would
                    # HOL-block the PE FIFO on that block's ~3us norm chain,
                    # so delay their first pop a few kt.
                    pop_delay = {3: 3, 7: 3}

                    def make_pops(bi):
                        # at most one filler per kt slot; leftovers drain at
                        # the block boundary, giving the PE guaranteed work
                        # across the norm chain so HAM stays warm.
                        def pops(kt, nkt):
                            q = queues[bi]
                            if q and kt >= pop_delay.get(bi, 0):
                                q.pop(0)()
                        return pops

                    for bi, (p, qc) in enumerate(blocks):
                        emit_block(p, qc, make_pops(bi),
                                   tail=(bi == len(blocks) - 1))
                        while queues[bi]:
                            queues[bi].pop(0)()
                        if p == 1:
                            units = [lambda qt=qt, oc=oc: c_unit(qt, oc)
                                     for qt in range(qc * 4, (qc + 1) * 4)
                                     for oc in range(2)]
                            for tgt, n in unit_route.get(bi, []):
                                queues[tgt].extend(units[:n])
                                units = units[n:]
                            for u in units:
                                u()

            if debug:
                for p in range(2):
                    nc.sync.dma_start(out=dbg["qT"][:, p, :], in_=qT_sb[p])
                    nc.sync.dma_start(out=dbg["kT"][:, p, :], in_=kT_sb[p])
                    nc.sync.dma_start(out=dbg["oT"][:, p, :], in_=outT_sb[p])
                for t in range(NKT):
                    nc.sync.dma_start(out=dbg["vv"][:, t, :], in_=v_sb[t])

    nc.compile()
    return nc


_cached = {}


def _get_nc(debug=False):
    key = bool(debug)
    if key not in _cached:
        _cached[key] = _build(debug)
    return _cached[key]


def _prep_inputs(x, w_q, b_q, w_k, w_v):
    tri = np.triu(np.ones((128, 128), np.float32)).astype(NPBF)
    wqT_f = np.ascontiguousarray(w_q.T).astype(NPBF)
    wkT_f = np.ascontiguousarray(w_k.T).astype(NPBF)
    wvT_f = np.ascontiguousarray(w_v.T).astype(NPBF)
    in_maps = []
    for core in range(N_CORES):
        b, hg = divmod(core, 4)
        cs = slice(hg * 256, (hg + 1) * 256)
        in_maps.append({
            "xT": np.ascontiguousarray(x[b].T).astype(NPBF),
            "wqT": np.ascontiguousarray(wqT_f[:, cs]),
            "wkT": np.ascontiguousarray(wkT_f[:, cs]),
            "wvT": np.ascontiguousarray(wvT_f[:, cs]),
            "bq2": np.ascontiguousarray(
                b_q[hg * 256:(hg + 1) * 256].reshape(2, 128).T.astype(np.float32)),
            "tri": tri,
        })
    return in_maps


def _numpy_reference(x, attention_mask, w_q, b_q, w_k, b_k, w_v, b_v, w_o, b_o):
    x = x.astype(np.float64)
    q = (x @ w_q.T + b_q).reshape(B, S, H, DK).transpose(0, 2, 1, 3)
    k = (x @ w_k.T + b_k).reshape(B, S, H, DK).transpose(0, 2, 1, 3)
    v = (x @ w_v.T + b_v).reshape(B, S, H, DK).transpose(0, 2, 1, 3)
    scores = np.einsum("bhqd,bhkd->bhqk", q, k) / np.sqrt(DK)
    causal = np.tril(np.ones((S, S), bool))
    mask = causal[None, None] & (attention_mask[:, None, None, :] != 0)
    scores = np.where(mask, scores, -np.inf)
    scores -= scores.max(-1, keepdims=True)
    e = np.exp(scores)
    attn = e / e.sum(-1, keepdims=True)
    out = np.einsum("bhqk,bhkd->bhqd", attn, v)
    out = out.transpose(0, 2, 1, 3).reshape(B, S, D)
    return (out @ w_o.T + b_o).astype(np.float32)


def kernel(x, attention_mask, w_q, b_q, w_k, b_k, w_v, b_v, w_o, b_o,
           _debug=False, _trace=False):
    x = np.asarray(x, np.float32)
    attention_mask = np.asarray(attention_mask)
    if not np.all(attention_mask != 0):
        return _numpy_reference(np.asarray(x), np.asarray(attention_mask),
                                *[np.asarray(a) for a in
                                  (w_q, b_q, w_k, b_k, w_v, b_v, w_o, b_o)])
    w_q, w_k, w_v, w_o = [np.asarray(w, np.float32) for w in (w_q, w_k, w_v, w_o)]
    b_q, b_k, b_v, b_o = [np.asarray(b, np.float32) for b in (b_q, b_k, b_v, b_o)]

    nc = _get_nc(_debug)
    in_maps = _prep_inputs(x, w_q, b_q, w_k, w_v)
    woT_f = np.ascontiguousarray(w_o.T).astype(NPBF)
    for core in range(N_CORES):
        hg = core % 4
        in_maps[core]["woT"] = np.ascontiguousarray(
            woT_f[hg * 256:(hg + 1) * 256, :])

    res = run_bass_kernel_spmd(nc, in_maps, list(range(N_CORES)), trace=_trace)
    const_row = (b_v @ w_o.T + b_o).astype(np.float32)
    y = np.zeros((B, S, D), np.float32)
    for core in range(N_CORES):
        b = core // 4
        y[b] += res.results[core]["y"].astype(np.float32)
    y += const_row
    if _debug or _trace:
        return y, res
    return y



# revision 71
# speedup vs baseline: 1.0071x; 1.0071x over previous
"""Multi-head causal self-attention (B=2, S=2048, D=1024, H=16) on 8 TRN2 cores.

Sharding: core = b*4 + hg  (b in {0,1} batch, hg in {0..3} head-group of 4 heads).
Per core: project qT/kT (pair-packed [128, S], bf16) and v ([S, 64] blocks, bf16),
compute transposed scores S^T = K Q^T per head (k on partitions), causal mask
added in PSUM via identity-matmul, exp on ScalarE (bf16 out), PV matmul with a
ones-column appended to V so row 64 of the accumulator is the softmax sum,
normalization via reciprocal + DMA partition-broadcast + tensor mul, then the
partial output projection. Host sums the 4 per-batch partials and adds
(b_v @ w_o.T + b_o); b_k is dropped (softmax is invariant to per-query
constants); b_q is applied on-device. Matmul operands are bf16 (fp32 moving
operand streams at half rate on TRN2); all accumulation is fp32 in PSUM.
"""

import numpy as np
import ml_dtypes

import concourse.bass as bass
import concourse.mybir as mybir
import concourse.tile as tile
from concourse import bacc
from concourse.bass_utils import run_bass_kernel_spmd

B, S, D, H, DK = 2, 2048, 1024, 16, 64
N_CORES = 8
F32 = mybir.dt.float32
BF16 = mybir.dt.bfloat16
NPBF = ml_dtypes.bfloat16
AF = mybir.ActivationFunctionType
NEG_BIG = -1.0e9


def _build(debug=False):
    nc = bacc.Bacc("TRN2", target_bir_lowering=False, debug=False,
                   num_devices=N_CORES)
    xT = nc.dram_tensor("xT", [D, S], BF16, kind="ExternalInput").ap()
    wqT = nc.dram_tensor("wqT", [D, 256], BF16, kind="ExternalInput").ap()
    wkT = nc.dram_tensor("wkT", [D, 256], BF16, kind="ExternalInput").ap()
    wvT = nc.dram_tensor("wvT", [D, 256], BF16, kind="ExternalInput").ap()
    woT = nc.dram_tensor("woT", [256, D], BF16, kind="ExternalInput").ap()
    bq2 = nc.dram_tensor("bq2", [128, 2], F32, kind="ExternalInput").ap()
    tri = nc.dram_tensor("tri", [128, 128], BF16, kind="ExternalInput").ap()
    y = nc.dram_tensor("y", [S, D], BF16, kind="ExternalOutput").ap()
    dbg = {}
    if debug:
        for nm, shp in [("qT", [128, 2, S]), ("kT", [128, 2, S]),
                        ("vv", [128, 16, 260]), ("oT", [128, 2, S])]:
            dbg[nm] = nc.dram_tensor(nm, shp, BF16, kind="ExternalOutput").ap()

    NQC = 4          # q-chunks of 512
    QC = 512
    NKT = S // 128   # k tiles

    with tile.TileContext(nc) as tc, \
            nc.allow_low_precision(reason="bf16 attention kernel"):
        with (
            tc.tile_pool(name="persist", bufs=1) as persist,
            tc.tile_pool(name="kqv", bufs=2) as kqv,
        ):
            qT_sb = [kqv.tile([128, S], BF16, tag="qT", name=f"qT{p}") for p in range(2)]
            kT_sb = [kqv.tile([128, S], BF16, tag="kT", name=f"kT{p}") for p in range(2)]
            v_sb = [persist.tile([128, 4 * 65], BF16, tag=f"v{t}", name=f"v{t}")
                    for t in range(NKT)]
            outT_sb = [persist.tile([128, S], BF16, tag=f"oT{p}", name=f"oTs{p}")
                       for p in range(2)]
            wo_sb = [persist.tile([128, D], BF16, tag=f"wo{p}", name=f"wo{p}")
                     for p in range(2)]
            tri_sb = persist.tile([128, 128], BF16, tag="tri")
            bq_sb = persist.tile([128, 2], F32, tag="bq")

            with (
                tc.tile_pool(name="xw", bufs=1) as xw,
                tc.tile_pool(name="ep", bufs=7) as ep,
                tc.tile_pool(name="rp", bufs=6) as rp,
            ):
                xt = [xw.tile([128, S], BF16, tag=f"x{c}", name=f"xt{c}") for c in range(8)]
                wq_sb = [xw.tile([128, 256], BF16, tag=f"wq{c}", name=f"wqs{c}") for c in range(8)]
                wk_sb = [xw.tile([128, 256], BF16, tag=f"wk{c}", name=f"wks{c}") for c in range(8)]
                wv_sb = [xw.tile([128, 256], BF16, tag=f"wv{c}", name=f"wvs{c}") for c in range(8)]
                # xt on the sync HWDGE queue; wq on the scalar HWDGE queue;
                # wk/wv interleaved on gpsimd so chunk c lands just before
                # the xt chunk that gates its preamble matmul.
                for c in range(8):
                    nc.sync.dma_start(out=xt[c], in_=xT[c * 128:(c + 1) * 128, :])
                for c in range(8):
                    nc.scalar.dma_start(out=wq_sb[c], in_=wqT[c * 128:(c + 1) * 128, :])
                for c in range(8):
                    nc.gpsimd.dma_start(out=wk_sb[c], in_=wkT[c * 128:(c + 1) * 128, :])
                    nc.gpsimd.dma_start(out=wv_sb[c], in_=wvT[c * 128:(c + 1) * 128, :])
                nc.gpsimd.dma_start(out=bq_sb, in_=bq2)
                nc.gpsimd.dma_start(out=tri_sb, in_=tri)
                for p in range(2):
                    nc.gpsimd.dma_start(out=wo_sb[p], in_=woT[p * 128:(p + 1) * 128, :])

                # trace-order schedule validator: a read before its write in
                # trace order silently reads garbage, so assert every block's
                # needs were emitted earlier.
                written = set()

                def qk_chain(p, j, which, pool):
                    written.add((which, p, j))
                    ps = pool.tile([128, QC], F32, tag="proj", name="ps")
                    w_sb = wq_sb if which == "q" else wk_sb
                    for c in range(8):
                        nc.tensor.matmul(
                            ps, w_sb[c][:, p * 128:(p + 1) * 128],
                            xt[c][:, j * QC:(j + 1) * QC],
                            start=(c == 0), stop=(c == 7))
                    if which == "q":
                        nc.vector.tensor_scalar_add(
                            qT_sb[p][:, j * QC:(j + 1) * QC], ps, bq_sb[:, p:p + 1])
                    else:
                        nc.vector.tensor_copy(kT_sb[p][:, j * QC:(j + 1) * QC], ps)

                def v_chain(t, pool):
                    written.add(("v", t))
                    ps_v = pool.tile([128, 256], F32, tag="proj", name="ps_v")
                    for c in range(8):
                        nc.tensor.matmul(
                            ps_v, xt[c][:, t * 128:(t + 1) * 128], wv_sb[c],
                            start=(c == 0), stop=(c == 7))
                    v_view = v_sb[t].rearrange("p (h w) -> p h w", w=65)
                    nc.vector.memset(v_view[:, :, 64:65], 1.0)
                    nc.vector.tensor_copy(
                        v_view[:, :, 0:64],
                        ps_v.rearrange("p (h w) -> p h w", w=64))

                def emit_norm(p_, q0_, o_ps_, tail=False):
                    written.add(("outT", p_, q0_ // QC))
                    # recip of the ones-row sum (copied to SBUF first —
                    # reciprocal_approx_fast from PSUM returns garbage),
                    # broadcast along partitions on GpSimd, fused multiply
                    # from PSUM into the bf16 outT tile. In the kernel tail
                    # (last block) the sums copy goes to the idle ScalarE and
                    # the multiply is split per q-tile so the output
                    # projection units start sooner.
                    if tail:
                        # 256-wide chunks: the first output-projection units
                        # unlock after one chunk-chain (~2us) instead of the
                        # full-width chain (~4us), keeping the PE-idle gap
                        # under the HAM re-throttle window.
                        for qq in range(2):
                            cs = slice(qq * 256, (qq + 1) * 256)
                            for s in range(2):
                                sums = rp.tile([1, 256], F32, tag="sums_t",
                                               name="sums_t")
                                nc.scalar.activation(
                                    sums, o_ps_[s][64:65, cs], AF.Copy)
                                recip = rp.tile([1, 256], F32, tag="recip_t",
                                                name="recip_t")
                                nc.vector.reciprocal_approx_fast(
                                    out=recip, in_=sums)
                                bc = rp.tile([64, 256], F32, tag="bc_t",
                                             name="bc_t")
                                nc.gpsimd.partition_broadcast(bc, recip)
                                nc.vector.tensor_mul(
                                    outT_sb[p_][s * 64:(s + 1) * 64,
                                                q0_ + qq * 256:q0_ + (qq + 1) * 256],
                                    o_ps_[s][0:64, cs], bc)
                        return
                    for s in range(2):
                        sums = rp.tile([1, QC], F32, tag="sums", name="sums")
                        nc.vector.tensor_copy(sums, o_ps_[s][64:65, :])
                        recip = rp.tile([1, QC], F32, tag="recip", name="recip")
                        nc.vector.reciprocal_approx_fast(out=recip, in_=sums)
                        bc = rp.tile([64, QC], F32, tag="bc", name="bc")
                        nc.gpsimd.partition_broadcast(bc, recip)
                        nc.vector.tensor_mul(
                            outT_sb[p_][s * 64:(s + 1) * 64, q0_:q0_ + QC],
                            o_ps_[s][0:64, :], bc)

                def emit_pv(p, o_ps, nkt, ent):
                    _kt, _e, _lo = ent
                    for s in range(2):
                        hb = 2 * p + s
                        nc.tensor.matmul(
                            o_ps[s][:, _lo:QC],
                            v_sb[_kt][:, hb * 65:(hb + 1) * 65],
                            _e[:, s * QC + _lo:(s + 1) * QC],
                            start=(_kt == 0), stop=(_kt == nkt - 1),
                            skip_group_check=True)

                def emit_block(p, qc, pops, tail=False):
                    assert ("q", p, qc) in written, (p, qc, "q")
                    for j in range(qc + 1):
                        assert ("k", p, j) in written, (p, qc, "k", j)
                    for t in range(4 * qc + 4):
                        assert ("v", t) in written, (p, qc, "v", t)
                    q0 = qc * QC
                    nkt = 4 * qc + 4
                    o_ps = [opp.tile([65, QC], F32, tag=f"o{s}", name=f"ops{s}")
                            for s in range(2)]
                    pend = []
                    for kt in range(nkt):
                        o = kt * 128 - q0
                        diag = o >= 0
                        lo = o if diag else 0
                        s_ab = sqp.tile([128, 2 * QC], F32, tag="sq", name="s_ab")
                        for s in range(2):
                            half = s * QC
                            nc.tensor.matmul(
                                s_ab[:, half + lo:half + QC],
                                kT_sb[p][s * 64:(s + 1) * 64,
                                         kt * 128:(kt + 1) * 128],
                                qT_sb[p][s * 64:(s + 1) * 64,
                                         q0 + lo:q0 + QC],
                                start=True, stop=True,
                                tile_position=(s * 64, 0),
                                skip_group_check=True)
                        e_ab = ep.tile([128, 2 * QC], BF16, tag="e", name="e_ab")
                        if diag:
                            # one ACTIVATE spanning both heads; cols
                            # [QC:QC+lo] hold exp(stale psum) and are
                            # never read downstream.
                            nc.scalar.activation(
                                e_ab[:, lo:2 * QC], s_ab[:, lo:2 * QC],
                                AF.Exp, scale=0.125)
                            for s in range(2):
                                nc.vector.tensor_mul(
                                    e_ab[:, s * QC + o:s * QC + o + 128],
                                    e_ab[:, s * QC + o:s * QC + o + 128],
                                    tri_sb)
                        else:
                            nc.scalar.activation(e_ab, s_ab, AF.Exp, scale=0.125)
                        # PV lags scores by 3 kt: exp(kt) gets ~3 PE rounds of
                        # slack, the first PV of a block lands after the prior
                        # block's norm has released the o_ps slots, and the
                        # end-of-block flush leaves a PE burst at the boundary.
                        if len(pend) == 4:
                            emit_pv(p, o_ps, nkt, pend.pop(0))
                        pops(kt, nkt)
                        pend.append((kt, e_ab, lo))
                    while pend:
                        emit_pv(p, o_ps, nkt, pend.pop(0))
                    emit_norm(p, q0, o_ps, tail=tail)

                # ---- preamble: pair-0 j0 q/k + v0-3, interleaved c-major so
                # every arriving x chunk unlocks 6 matmuls. (Extending this
                # with pair-1 j0 measured 200us vs 164.5 — do not retry.) ----
                with tc.tile_pool(name="ppsA", bufs=6, space="PSUM") as ppsA:
                    ps_q0 = ppsA.tile([128, QC], F32, tag="projA", name="ps_q0")
                    ps_k0 = ppsA.tile([128, QC], F32, tag="projA", name="ps_k0")
                    ps_vh = [ppsA.tile([128, 256], F32, tag="projA",
                                       name=f"ps_vh{t}") for t in range(4)]
                    for c in range(8):
                        nc.tensor.matmul(
                            ps_q0, wq_sb[c][:, 0:128], xt[c][:, 0:QC],
                            start=(c == 0), stop=(c == 7))
                        nc.tensor.matmul(
                            ps_k0, wk_sb[c][:, 0:128], xt[c][:, 0:QC],
                            start=(c == 0), stop=(c == 7))
                        for t in range(4):
                            nc.tensor.matmul(
                                ps_vh[t], xt[c][:, t * 128:(t + 1) * 128],
                                wv_sb[c], start=(c == 0), stop=(c == 7))
                    written.update({("q", 0, 0), ("k", 0, 0),
                                    ("v", 0), ("v", 1), ("v", 2), ("v", 3)})
                    nc.vector.tensor_scalar_add(
                        qT_sb[0][:, 0:QC], ps_q0, bq_sb[:, 0:1])
                    nc.vector.tensor_copy(kT_sb[0][:, 0:QC], ps_k0)
                    for t in range(4):
                        v_view = v_sb[t].rearrange("p (h w) -> p h w", w=65)
                        nc.vector.memset(v_view[:, :, 64:65], 1.0)
                        nc.vector.tensor_copy(
                            v_view[:, :, 0:64],
                            ps_vh[t].rearrange("p (h w) -> p h w", w=64))
                # ---- interleaved pair-0 / pair-1 attention blocks, with
                # projection chains and output-projection units as fillers ----
                with (
                    tc.tile_pool(name="sq", bufs=2, space="PSUM") as sqp,
                    tc.tile_pool(name="ops", bufs=1, space="PSUM") as opp,
                    tc.tile_pool(name="aux", bufs=2, space="PSUM") as aux,
                    tc.tile_pool(name="fsb", bufs=6) as fsb,
                ):
                    f_hold = {}

                    def c_unit(qt, oc):
                        for p_ in range(2):
                            assert ("outT", p_, qt // 4) in written, (qt, oc, p_)
                        f_ps = aux.tile([128, 512], F32, tag="proj", name="f_ps")
                        for p in range(2):
                            nc.tensor.matmul(
                                f_ps, outT_sb[p][:, qt * 128:(qt + 1) * 128],
                                wo_sb[p][:, oc * 512:(oc + 1) * 512],
                                start=(p == 0), stop=(p == 1))
                        if oc == 0:
                            f_sb = fsb.tile([128, 1024], BF16, tag="f",
                                            name="f_sb")
                            f_hold[qt] = f_sb
                        else:
                            f_sb = f_hold.pop(qt)
                        nc.vector.tensor_copy(
                            f_sb[:, oc * 512:(oc + 1) * 512], f_ps)
                        if oc == 1:
                            nc.sync.dma_start(
                                out=y[qt * 128:(qt + 1) * 128, :], in_=f_sb)

                    def qk(p, j, w):
                        return lambda: qk_chain(p, j, w, aux)

                    def vch(t):
                        return lambda: v_chain(t, aux)

                    # Block order mixes ScalarE-heavy attention with PE-heavy
                    # projections and ends on the smallest block (1,1) to
                    # shrink the tail.
                    blocks = [(0, 0), (0, 1), (1, 0), (0, 2),
                              (1, 1), (0, 3), (1, 2), (1, 3)]
                    # deadline table: block (p,qc) needs its pair's k chunks
                    # j<=qc and q chunk j=qc written in an EARLIER block (the
                    # preamble covers pair-0 j0 and v0-3).
                    # fillers pop as late as their deadline allows so the
                    # ScalarE-paced late blocks keep the PE warm.
                    queues = {i: [] for i in range(8)}
                    queues[0] = [qk(0, 1, "q"), qk(0, 1, "k"),
                                 vch(4), vch(5), vch(6), vch(7)]
                    queues[1] = [qk(1, 0, "k"), qk(1, 0, "q"),
                                 qk(0, 2, "q"), qk(0, 2, "k"),
                                 vch(8), vch(9), vch(10), vch(11)]
                    queues[2] = [qk(1, 1, "k"), qk(1, 1, "q"), vch(12), vch(13)]
                    queues[4] = [qk(0, 3, "q"), qk(0, 3, "k"), vch(14), vch(15)]
                    queues[5] = [qk(1, 3, "k"), qk(1, 3, "q"),
                                 qk(1, 2, "k"), qk(1, 2, "q")]
                    # c_unit routing: units for qc_j may only run after BOTH
                    # (0,j) and (1,j) blocks have produced outT for qt range.
                    unit_route = {2: [(3, 8)], 4: [(5, 4), (6, 4)], 6: [(7, 8)]}

                    # queues 3 and 7 hold only c_units produced by the
                    # immediately-preceding block; popping them at kt=0 would
                    # HOL-block the PE FIFO on that block's ~3us norm chain,
                    # so delay their first pop a few kt.
                    pop_delay = {3: 3, 7: 3}

                    def make_pops(bi):
                        # at most one filler per kt slot; leftovers drain at
                        # the block boundary, giving the PE guaranteed work
                        # across the norm chain so HAM stays warm.
                        def pops(kt, nkt):
                            q = queues[bi]
                            if q and kt >= pop_delay.get(bi, 0):
                                q.pop(0)()
                        return pops

                    for bi, (p, qc) in enumerate(blocks):
                        emit_block(p, qc, make_pops(bi),
                                   tail=(bi == len(blocks) - 1))
                        while queues[bi]:
                            queues[bi].pop(0)()
                        if p == 1:
                            units = [lambda qt=qt, oc=oc: c_unit(qt, oc)
                                     for qt in range(qc * 4, (qc + 1) * 4)
                                     for oc in range(2)]
                            for tgt, n in unit_route.get(bi, []):
                                queues[tgt].extend(units[:n])
                                units = units[n:]
                            for u in units:
                                u()

            if debug:
                for p in range(2):
                    nc.sync.dma_start(out=dbg["qT"][:, p, :], in_=qT_sb[p])
                    nc.sync.dma_start(out=dbg["kT"][:, p, :], in_=kT_sb[p])
                    nc.sync.dma_start(out=dbg["oT"][:, p, :], in_=outT_sb[p])
                for t in range(NKT):
                    nc.sync.dma_start(out=dbg["vv"][:, t, :], in_=v_sb[t])

    nc.compile()
    return nc


_cached = {}


def _get_nc(debug=False):
    key = bool(debug)
    if key not in _cached:
        _cached[key] = _build(debug)
    return _cached[key]


def _prep_inputs(x, w_q, b_q, w_k, w_v):
    tri = np.triu(np.ones((128, 128), np.float32)).astype(NPBF)
    wqT_f = np.ascontiguousarray(w_q.T).astype(NPBF)
    wkT_f = np.ascontiguousarray(w_k.T).astype(NPBF)
    wvT_f = np.ascontiguousarray(w_v.T).astype(NPBF)
    in_maps = []
    for core in range(N_CORES):
        b, hg = divmod(core, 4)
        cs = slice(hg * 256, (hg + 1) * 256)
        in_maps.append({
            "xT": np.ascontiguousarray(x[b].T).astype(NPBF),
            "wqT": np.ascontiguousarray(wqT_f[:, cs]),
            "wkT": np.ascontiguousarray(wkT_f[:, cs]),
            "wvT": np.ascontiguousarray(wvT_f[:, cs]),
            "bq2": np.ascontiguousarray(
                b_q[hg * 256:(hg + 1) * 256].reshape(2, 128).T.astype(np.float32)),
            "tri": tri,
        })
    return in_maps


def _numpy_reference(x, attention_mask, w_q, b_q, w_k, b_k, w_v, b_v, w_o, b_o):
    x = x.astype(np.float64)
    q = (x @ w_q.T + b_q).reshape(B, S, H, DK).transpose(0, 2, 1, 3)
    k = (x @ w_k.T + b_k).reshape(B, S, H, DK).transpose(0, 2, 1, 3)
    v = (x @ w_v.T + b_v).reshape(B, S, H, DK).transpose(0, 2, 1, 3)
    scores = np.einsum("bhqd,bhkd->bhqk", q, k) / np.sqrt(DK)
    causal = np.tril(np.ones((S, S), bool))
    mask = causal[None, None] & (attention_mask[:, None, None, :] != 0)
    scores = np.where(mask, scores, -np.inf)
    scores -= scores.max(-1, keepdims=True)
    e = np.exp(scores)
    attn = e / e.sum(-1, keepdims=True)
    out = np.einsum("bhqk,bhkd->bhqd", attn, v)
    out = out.transpose(0, 2, 1, 3).reshape(B, S, D)
    return (out @ w_o.T + b_o).astype(np.float32)


def kernel(x, attention_mask, w_q, b_q, w_k, b_k, w_v, b_v, w_o, b_o,
           _debug=False, _trace=False):
    x = np.asarray(x, np.float32)
    attention_mask = np.asarray(attention_mask)
    if not np.all(attention_mask != 0):
        return _numpy_reference(np.asarray(x), np.asarray(attention_mask),
                                *[np.asarray(a) for a in
                                  (w_q, b_q, w_k, b_k, w_v, b_v, w_o, b_o)])
    w_q, w_k, w_v, w_o = [np.asarray(w, np.float32) for w in (w_q, w_k, w_v, w_o)]
    b_q, b_k, b_v, b_o = [np.asarray(b, np.float32) for b in (b_q, b_k, b_v, b_o)]

    nc = _get_nc(_debug)
    in_maps = _prep_inputs(x, w_q, b_q, w_k, w_v)
    woT_f = np.ascontiguousarray(w_o.T).astype(NPBF)
    for core in range(N_CORES):
        hg = core % 4
        in_maps[core]["woT"] = np.ascontiguousarray(
            woT_f[hg * 256:(hg + 1) * 256, :])

    res = run_bass_kernel_spmd(nc, in_maps, list(range(N_CORES)), trace=_trace)
    const_row = (b_v @ w_o.T + b_o).astype(np.float32)
    y = np.zeros((B, S, D), np.float32)
    for core in range(N_CORES):
        b = core // 4
        y[b] += res.results[core]["y"].astype(np.float32)
    y += const_row
    if _debug or _trace:
        return y, res
    return y



# revision 74
# speedup vs baseline: 1.0130x; 1.0059x over previous
"""Multi-head causal self-attention (B=2, S=2048, D=1024, H=16) on 8 TRN2 cores.

Sharding: core = b*4 + hg  (b in {0,1} batch, hg in {0..3} head-group of 4 heads).
Per core: project qT/kT (pair-packed [128, S], bf16) and v ([S, 64] blocks, bf16),
compute transposed scores S^T = K Q^T per head (k on partitions), causal mask
added in PSUM via identity-matmul, exp on ScalarE (bf16 out), PV matmul with a
ones-column appended to V so row 64 of the accumulator is the softmax sum,
normalization via reciprocal + DMA partition-broadcast + tensor mul, then the
partial output projection. Host sums the 4 per-batch partials and adds
(b_v @ w_o.T + b_o); b_k is dropped (softmax is invariant to per-query
constants); b_q is applied on-device. Matmul operands are bf16 (fp32 moving
operand streams at half rate on TRN2); all accumulation is fp32 in PSUM.
"""

import numpy as np
import ml_dtypes

import concourse.bass as bass
import concourse.mybir as mybir
import concourse.tile as tile
from concourse import bacc
from concourse.bass_utils import run_bass_kernel_spmd

B, S, D, H, DK = 2, 2048, 1024, 16, 64
N_CORES = 8
F32 = mybir.dt.float32
BF16 = mybir.dt.bfloat16
NPBF = ml_dtypes.bfloat16
AF = mybir.ActivationFunctionType
NEG_BIG = -1.0e9


def _build(debug=False):
    nc = bacc.Bacc("TRN2", target_bir_lowering=False, debug=False,
                   num_devices=N_CORES)
    xT = nc.dram_tensor("xT", [D, S], BF16, kind="ExternalInput").ap()
    wqT = nc.dram_tensor("wqT", [D, 256], BF16, kind="ExternalInput").ap()
    wkT = nc.dram_tensor("wkT", [D, 256], BF16, kind="ExternalInput").ap()
    wvT = nc.dram_tensor("wvT", [D, 256], BF16, kind="ExternalInput").ap()
    woT = nc.dram_tensor("woT", [256, D], BF16, kind="ExternalInput").ap()
    bq2 = nc.dram_tensor("bq2", [128, 2], F32, kind="ExternalInput").ap()
    tri = nc.dram_tensor("tri", [128, 128], BF16, kind="ExternalInput").ap()
    y = nc.dram_tensor("y", [S, D], BF16, kind="ExternalOutput").ap()
    dbg = {}
    if debug:
        for nm, shp in [("qT", [128, 2, S]), ("kT", [128, 2, S]),
                        ("vv", [128, 16, 260]), ("oT", [128, 2, S])]:
            dbg[nm] = nc.dram_tensor(nm, shp, BF16, kind="ExternalOutput").ap()

    NQC = 4          # q-chunks of 512
    QC = 512
    NKT = S // 128   # k tiles

    with tile.TileContext(nc) as tc, \
            nc.allow_low_precision(reason="bf16 attention kernel"):
        with (
            tc.tile_pool(name="persist", bufs=1) as persist,
            tc.tile_pool(name="kqv", bufs=2) as kqv,
        ):
            qT_sb = [kqv.tile([128, S], BF16, tag="qT", name=f"qT{p}") for p in range(2)]
            kT_sb = [kqv.tile([128, S], BF16, tag="kT", name=f"kT{p}") for p in range(2)]
            v_sb = [persist.tile([128, 4 * 65], BF16, tag=f"v{t}", name=f"v{t}")
                    for t in range(NKT)]
            outT_sb = [persist.tile([128, S], BF16, tag=f"oT{p}", name=f"oTs{p}")
                       for p in range(2)]
            wo_sb = [persist.tile([128, D], BF16, tag=f"wo{p}", name=f"wo{p}")
                     for p in range(2)]
            tri_sb = persist.tile([128, 128], BF16, tag="tri")
            bq_sb = persist.tile([128, 2], F32, tag="bq")

            with (
                tc.tile_pool(name="xw", bufs=1) as xw,
                tc.tile_pool(name="ep", bufs=7) as ep,
                tc.tile_pool(name="rp", bufs=6) as rp,
            ):
                xt = [xw.tile([128, S], BF16, tag=f"x{c}", name=f"xt{c}") for c in range(8)]
                wq_sb = [xw.tile([128, 256], BF16, tag=f"wq{c}", name=f"wqs{c}") for c in range(8)]
                wk_sb = [xw.tile([128, 256], BF16, tag=f"wk{c}", name=f"wks{c}") for c in range(8)]
                wv_sb = [xw.tile([128, 256], BF16, tag=f"wv{c}", name=f"wvs{c}") for c in range(8)]
                # xt on the sync HWDGE queue; wq on the scalar HWDGE queue;
                # wk/wv interleaved on gpsimd so chunk c lands just before
                # the xt chunk that gates its preamble matmul.
                for c in range(8):
                    nc.sync.dma_start(out=xt[c], in_=xT[c * 128:(c + 1) * 128, :])
                for c in range(8):
                    nc.scalar.dma_start(out=wq_sb[c], in_=wqT[c * 128:(c + 1) * 128, :])
                for c in range(8):
                    nc.gpsimd.dma_start(out=wk_sb[c], in_=wkT[c * 128:(c + 1) * 128, :])
                    nc.gpsimd.dma_start(out=wv_sb[c], in_=wvT[c * 128:(c + 1) * 128, :])
                nc.gpsimd.dma_start(out=bq_sb, in_=bq2)
                nc.gpsimd.dma_start(out=tri_sb, in_=tri)
                for p in range(2):
                    nc.gpsimd.dma_start(out=wo_sb[p], in_=woT[p * 128:(p + 1) * 128, :])

                # trace-order schedule validator: a read before its write in
                # trace order silently reads garbage, so assert every block's
                # needs were emitted earlier.
                written = set()

                def qk_chain(p, j, which, pool):
                    written.add((which, p, j))
                    ps = pool.tile([128, QC], F32, tag="proj", name="ps")
                    w_sb = wq_sb if which == "q" else wk_sb
                    for c in range(8):
                        nc.tensor.matmul(
                            ps, w_sb[c][:, p * 128:(p + 1) * 128],
                            xt[c][:, j * QC:(j + 1) * QC],
                            start=(c == 0), stop=(c == 7))
                    if which == "q":
                        nc.vector.tensor_scalar_add(
                            qT_sb[p][:, j * QC:(j + 1) * QC], ps, bq_sb[:, p:p + 1])
                    else:
                        nc.vector.tensor_copy(kT_sb[p][:, j * QC:(j + 1) * QC], ps)

                def v_chain(t, pool):
                    written.add(("v", t))
                    ps_v = pool.tile([128, 256], F32, tag="proj", name="ps_v")
                    for c in range(8):
                        nc.tensor.matmul(
                            ps_v, xt[c][:, t * 128:(t + 1) * 128], wv_sb[c],
                            start=(c == 0), stop=(c == 7))
                    v_view = v_sb[t].rearrange("p (h w) -> p h w", w=65)
                    nc.vector.memset(v_view[:, :, 64:65], 1.0)
                    nc.vector.tensor_copy(
                        v_view[:, :, 0:64],
                        ps_v.rearrange("p (h w) -> p h w", w=64))

                def emit_norm(p_, q0_, o_ps_, tail=False):
                    written.add(("outT", p_, q0_ // QC))
                    # recip of the ones-row sum (copied to SBUF first —
                    # reciprocal_approx_fast from PSUM returns garbage),
                    # broadcast along partitions on GpSimd, fused multiply
                    # from PSUM into the bf16 outT tile. In the kernel tail
                    # (last block) the sums copy goes to the idle ScalarE and
                    # the multiply is split per q-tile so the output
                    # projection units start sooner.
                    if tail:
                        # 256-wide chunks: the first output-projection units
                        # unlock after one chunk-chain (~2us) instead of the
                        # full-width chain (~4us), keeping the PE-idle gap
                        # under the HAM re-throttle window.
                        for qq in range(2):
                            cs = slice(qq * 256, (qq + 1) * 256)
                            for s in range(2):
                                sums = rp.tile([1, 256], F32, tag="sums_t",
                                               name="sums_t")
                                nc.scalar.activation(
                                    sums, o_ps_[s][64:65, cs], AF.Copy)
                                recip = rp.tile([1, 256], F32, tag="recip_t",
                                                name="recip_t")
                                nc.vector.reciprocal_approx_fast(
                                    out=recip, in_=sums)
                                bc = rp.tile([64, 256], F32, tag="bc_t",
                                             name="bc_t")
                                nc.gpsimd.partition_broadcast(bc, recip)
                                nc.vector.tensor_mul(
                                    outT_sb[p_][s * 64:(s + 1) * 64,
                                                q0_ + qq * 256:q0_ + (qq + 1) * 256],
                                    o_ps_[s][0:64, cs], bc)
                        return
                    for s in range(2):
                        sums = rp.tile([1, QC], F32, tag="sums", name="sums")
                        nc.vector.tensor_copy(sums, o_ps_[s][64:65, :])
                        recip = rp.tile([1, QC], F32, tag="recip", name="recip")
                        nc.vector.reciprocal_approx_fast(out=recip, in_=sums)
                        bc = rp.tile([64, QC], F32, tag="bc", name="bc")
                        nc.gpsimd.partition_broadcast(bc, recip)
                        nc.vector.tensor_mul(
                            outT_sb[p_][s * 64:(s + 1) * 64, q0_:q0_ + QC],
                            o_ps_[s][0:64, :], bc)

                def emit_pv(p, o_ps, nkt, ent):
                    _kt, _e, _lo = ent
                    for s in range(2):
                        hb = 2 * p + s
                        nc.tensor.matmul(
                            o_ps[s][:, _lo:QC],
                            v_sb[_kt][:, hb * 65:(hb + 1) * 65],
                            _e[:, s * QC + _lo:(s + 1) * QC],
                            start=(_kt == 0), stop=(_kt == nkt - 1),
                            skip_group_check=True)

                def emit_block(p, qc, pops, tail=False):
                    assert ("q", p, qc) in written, (p, qc, "q")
                    for j in range(qc + 1):
                        assert ("k", p, j) in written, (p, qc, "k", j)
                    for t in range(4 * qc + 4):
                        assert ("v", t) in written, (p, qc, "v", t)
                    q0 = qc * QC
                    nkt = 4 * qc + 4
                    o_ps = [opp.tile([65, QC], F32, tag=f"o{s}", name=f"ops{s}")
                            for s in range(2)]
                    pend = []
                    for kt in range(nkt):
                        o = kt * 128 - q0
                        diag = o >= 0
                        lo = o if diag else 0
                        s_ab = sqp.tile([128, 2 * QC], F32, tag="sq", name="s_ab")
                        for s in range(2):
                            half = s * QC
                            nc.tensor.matmul(
                                s_ab[:, half + lo:half + QC],
                                kT_sb[p][s * 64:(s + 1) * 64,
                                         kt * 128:(kt + 1) * 128],
                                qT_sb[p][s * 64:(s + 1) * 64,
                                         q0 + lo:q0 + QC],
                                start=True, stop=True,
                                tile_position=(s * 64, 0),
                                skip_group_check=True)
                        e_ab = ep.tile([128, 2 * QC], BF16, tag="e", name="e_ab")
                        if diag:
                            # one ACTIVATE spanning both heads; cols
                            # [QC:QC+lo] hold exp(stale psum) and are
                            # never read downstream.
                            nc.scalar.activation(
                                e_ab[:, lo:2 * QC], s_ab[:, lo:2 * QC],
                                AF.Exp, scale=0.125)
                            for s in range(2):
                                nc.vector.tensor_mul(
                                    e_ab[:, s * QC + o:s * QC + o + 128],
                                    e_ab[:, s * QC + o:s * QC + o + 128],
                                    tri_sb)
                        else:
                            nc.scalar.activation(e_ab, s_ab, AF.Exp, scale=0.125)
                        # PV lags scores by 3 kt: exp(kt) gets ~3 PE rounds of
                        # slack, the first PV of a block lands after the prior
                        # block's norm has released the o_ps slots, and the
                        # end-of-block flush leaves a PE burst at the boundary.
                        if len(pend) == 4:
                            emit_pv(p, o_ps, nkt, pend.pop(0))
                        pops(kt, nkt)
                        pend.append((kt, e_ab, lo))
                    while pend:
                        emit_pv(p, o_ps, nkt, pend.pop(0))
                    emit_norm(p, q0, o_ps, tail=tail)

                # ---- preamble: pair-0 j0 q/k + v0-3, interleaved c-major so
                # every arriving x chunk unlocks 6 matmuls. (Extending this
                # with pair-1 j0 measured 200us vs 164.5 — do not retry.) ----
                with tc.tile_pool(name="ppsA", bufs=6, space="PSUM") as ppsA:
                    ps_q0 = ppsA.tile([128, QC], F32, tag="projA", name="ps_q0")
                    ps_k0 = ppsA.tile([128, QC], F32, tag="projA", name="ps_k0")
                    ps_vh = [ppsA.tile([128, 256], F32, tag="projA",
                                       name=f"ps_vh{t}") for t in range(4)]
                    for c in range(8):
                        nc.tensor.matmul(
                            ps_q0, wq_sb[c][:, 0:128], xt[c][:, 0:QC],
                            start=(c == 0), stop=(c == 7))
                        nc.tensor.matmul(
                            ps_k0, wk_sb[c][:, 0:128], xt[c][:, 0:QC],
                            start=(c == 0), stop=(c == 7))
                        for t in range(4):
                            nc.tensor.matmul(
                                ps_vh[t], xt[c][:, t * 128:(t + 1) * 128],
                                wv_sb[c], start=(c == 0), stop=(c == 7))
                    written.update({("q", 0, 0), ("k", 0, 0),
                                    ("v", 0), ("v", 1), ("v", 2), ("v", 3)})
                    # one-time startup copy: kT on ScalarE (its queue is empty
                    # until the first exp), first k-tile's columns first, in
                    # parallel with the DVE qT bias-add.
                    nc.scalar.activation(
                        kT_sb[0][:, 0:128], ps_k0[:, 0:128], AF.Copy)
                    nc.vector.tensor_scalar_add(
                        qT_sb[0][:, 0:QC], ps_q0, bq_sb[:, 0:1])
                    nc.scalar.activation(
                        kT_sb[0][:, 128:QC], ps_k0[:, 128:QC], AF.Copy)
                    for t in range(4):
                        v_view = v_sb[t].rearrange("p (h w) -> p h w", w=65)
                        nc.vector.memset(v_view[:, :, 64:65], 1.0)
                        nc.vector.tensor_copy(
                            v_view[:, :, 0:64],
                            ps_vh[t].rearrange("p (h w) -> p h w", w=64))
                # ---- interleaved pair-0 / pair-1 attention blocks, with
                # projection chains and output-projection units as fillers ----
                with (
                    tc.tile_pool(name="sq", bufs=2, space="PSUM") as sqp,
                    tc.tile_pool(name="ops", bufs=1, space="PSUM") as opp,
                    tc.tile_pool(name="aux", bufs=2, space="PSUM") as aux,
                    tc.tile_pool(name="fsb", bufs=4) as fsb,
                ):
                    f_hold = {}

                    def c_unit(qt, oc):
                        for p_ in range(2):
                            assert ("outT", p_, qt // 4) in written, (qt, oc, p_)
                        f_ps = aux.tile([128, 512], F32, tag="proj", name="f_ps")
                        for p in range(2):
                            nc.tensor.matmul(
                                f_ps, outT_sb[p][:, qt * 128:(qt + 1) * 128],
                                wo_sb[p][:, oc * 512:(oc + 1) * 512],
                                start=(p == 0), stop=(p == 1))
                        if oc == 0:
                            f_sb = fsb.tile([128, 1024], BF16, tag="f",
                                            name="f_sb")
                            f_hold[qt] = f_sb
                        else:
                            f_sb = f_hold.pop(qt)
                        nc.vector.tensor_copy(
                            f_sb[:, oc * 512:(oc + 1) * 512], f_ps)
                        if oc == 1:
                            nc.sync.dma_start(
                                out=y[qt * 128:(qt + 1) * 128, :], in_=f_sb)

                    def qk(p, j, w):
                        return lambda: qk_chain(p, j, w, aux)

                    def vch(t):
                        return lambda: v_chain(t, aux)

                    # Block order mixes ScalarE-heavy attention with PE-heavy
                    # projections and ends on the smallest block (1,1) to
                    # shrink the tail.
                    blocks = [(0, 0), (0, 1), (1, 0), (0, 2),
                              (1, 1), (0, 3), (1, 2), (1, 3)]
                    # deadline table: block (p,qc) needs its pair's k chunks
                    # j<=qc and q chunk j=qc written in an EARLIER block (the
                    # preamble covers pair-0 j0 and v0-3).
                    # fillers pop as late as their deadline allows so the
                    # ScalarE-paced late blocks keep the PE warm.
                    queues = {i: [] for i in range(8)}
                    queues[0] = [qk(0, 1, "q"), qk(0, 1, "k"),
                                 vch(4), vch(5), vch(6), vch(7)]
                    queues[1] = [qk(1, 0, "k"), qk(1, 0, "q"),
                                 qk(0, 2, "q"), qk(0, 2, "k"),
                                 vch(8), vch(9), vch(10), vch(11)]
                    queues[2] = [qk(1, 1, "k"), qk(1, 1, "q"), vch(12), vch(13)]
                    queues[4] = [qk(0, 3, "q"), qk(0, 3, "k"), vch(14), vch(15),
                                 qk(1, 2, "k"), qk(1, 2, "q")]
                    queues[5] = [qk(1, 3, "k"), qk(1, 3, "q")]
                    # c_unit routing: units for qc_j may only run after BOTH
                    # (0,j) and (1,j) blocks have produced outT for qt range.
                    unit_route = {2: [(3, 8)], 4: [(5, 4), (6, 4)], 6: [(7, 8)]}

                    # queues 3 and 7 hold only c_units produced by the
                    # immediately-preceding block; popping them at kt=0 would
                    # HOL-block the PE FIFO on that block's ~3us norm chain,
                    # so delay their first pop a few kt.
                    pop_delay = {3: 3, 7: 3}

                    def make_pops(bi):
                        # at most one filler per kt slot; leftovers drain at
                        # the block boundary, giving the PE guaranteed work
                        # across the norm chain so HAM stays warm.
                        def pops(kt, nkt):
                            q = queues[bi]
                            if q and kt >= pop_delay.get(bi, 0):
                                q.pop(0)()
                        return pops

                    for bi, (p, qc) in enumerate(blocks):
                        emit_block(p, qc, make_pops(bi),
                                   tail=(bi == len(blocks) - 1))
                        while queues[bi]:
                            queues[bi].pop(0)()
                        if p == 1:
                            units = [lambda qt=qt, oc=oc: c_unit(qt, oc)
                                     for qt in range(qc * 4, (qc + 1) * 4)
                                     for oc in range(2)]
                            for tgt, n in unit_route.get(bi, []):
                                queues[tgt].extend(units[:n])
                                units = units[n:]
                            for u in units:
                                u()

            if debug:
                for p in range(2):
                    nc.sync.dma_start(out=dbg["qT"][:, p, :], in_=qT_sb[p])
                    nc.sync.dma_start(out=dbg["kT"][:, p, :], in_=kT_sb[p])
                    nc.sync.dma_start(out=dbg["oT"][:, p, :], in_=outT_sb[p])
                for t in range(NKT):
                    nc.sync.dma_start(out=dbg["vv"][:, t, :], in_=v_sb[t])

    nc.compile()
    return nc


_cached = {}


def _get_nc(debug=False):
    key = bool(debug)
    if key not in _cached:
        _cached[key] = _build(debug)
    return _cached[key]


def _prep_inputs(x, w_q, b_q, w_k, w_v):
    tri = np.triu(np.ones((128, 128), np.float32)).astype(NPBF)
    wqT_f = np.ascontiguousarray(w_q.T).astype(NPBF)
    wkT_f = np.ascontiguousarray(w_k.T).astype(NPBF)
    wvT_f = np.ascontiguousarray(w_v.T).astype(NPBF)
    in_maps = []
    for core in range(N_CORES):
        b, hg = divmod(core, 4)
        cs = slice(hg * 256, (hg + 1) * 256)
        in_maps.append({
            "xT": np.ascontiguousarray(x[b].T).astype(NPBF),
            "wqT": np.ascontiguousarray(wqT_f[:, cs]),
            "wkT": np.ascontiguousarray(wkT_f[:, cs]),
            "wvT": np.ascontiguousarray(wvT_f[:, cs]),
            "bq2": np.ascontiguousarray(
                b_q[hg * 256:(hg + 1) * 256].reshape(2, 128).T.astype(np.float32)),
            "tri": tri,
        })
    return in_maps


def _numpy_reference(x, attention_mask, w_q, b_q, w_k, b_k, w_v, b_v, w_o, b_o):
    x = x.astype(np.float64)
    q = (x @ w_q.T + b_q).reshape(B, S, H, DK).transpose(0, 2, 1, 3)
    k = (x @ w_k.T + b_k).reshape(B, S, H, DK).transpose(0, 2, 1, 3)
    v = (x @ w_v.T + b_v).reshape(B, S, H, DK).transpose(0, 2, 1, 3)
    scores = np.einsum("bhqd,bhkd->bhqk", q, k) / np.sqrt(DK)
    causal = np.tril(np.ones((S, S), bool))
    mask = causal[None, None] & (attention_mask[:, None, None, :] != 0)
    scores = np.where(mask, scores, -np.inf)
    scores -= scores.max(-1, keepdims=True)
    e = np.exp(scores)
    attn = e / e.sum(-1, keepdims=True)
    out = np.einsum("bhqk,bhkd->bhqd", attn, v)
    out = out.transpose(0, 2, 1, 3).reshape(B, S, D)
    return (out @ w_o.T + b_o).astype(np.float32)


def kernel(x, attention_mask, w_q, b_q, w_k, b_k, w_v, b_v, w_o, b_o,
           _debug=False, _trace=False):
    x = np.asarray(x, np.float32)
    attention_mask = np.asarray(attention_mask)
    if not np.all(attention_mask != 0):
        return _numpy_reference(np.asarray(x), np.asarray(attention_mask),
                                *[np.asarray(a) for a in
                                  (w_q, b_q, w_k, b_k, w_v, b_v, w_o, b_o)])
    w_q, w_k, w_v, w_o = [np.asarray(w, np.float32) for w in (w_q, w_k, w_v, w_o)]
    b_q, b_k, b_v, b_o = [np.asarray(b, np.float32) for b in (b_q, b_k, b_v, b_o)]

    nc = _get_nc(_debug)
    in_maps = _prep_inputs(x, w_q, b_q, w_k, w_v)
    woT_f = np.ascontiguousarray(w_o.T).astype(NPBF)
    for core in range(N_CORES):
        hg = core % 4
        in_maps[core]["woT"] = np.ascontiguousarray(
            woT_f[hg * 256:(hg + 1) * 256, :])

    res = run_bass_kernel_spmd(nc, in_maps, list(range(N_CORES)), trace=_trace)
    const_row = (b_v @ w_o.T + b_o).astype(np.float32)
    y = np.zeros((B, S, D), np.float32)
    for core in range(N_CORES):
        b = core // 4
        y[b] += res.results[core]["y"].astype(np.float32)
    y += const_row
    if _debug or _trace:
        return y, res
    return y



# revision 75
# speedup vs baseline: 1.0276x; 1.0144x over previous
"""Multi-head causal self-attention (B=2, S=2048, D=1024, H=16) on 8 TRN2 cores.

Sharding: core = b*4 + hg  (b in {0,1} batch, hg in {0..3} head-group of 4 heads).
Per core: project qT/kT (pair-packed [128, S], bf16) and v ([S, 64] blocks, bf16),
compute transposed scores S^T = K Q^T per head (k on partitions), causal mask
added in PSUM via identity-matmul, exp on ScalarE (bf16 out), PV matmul with a
ones-column appended to V so row 64 of the accumulator is the softmax sum,
normalization via reciprocal + DMA partition-broadcast + tensor mul, then the
partial output projection. Host sums the 4 per-batch partials and adds
(b_v @ w_o.T + b_o); b_k is dropped (softmax is invariant to per-query
constants); b_q is applied on-device. Matmul operands are bf16 (fp32 moving
operand streams at half rate on TRN2); all accumulation is fp32 in PSUM.
"""

import numpy as np
import ml_dtypes

import concourse.bass as bass
import concourse.mybir as mybir
import concourse.tile as tile
from concourse import bacc
from concourse.bass_utils import run_bass_kernel_spmd

B, S, D, H, DK = 2, 2048, 1024, 16, 64
N_CORES = 8
F32 = mybir.dt.float32
BF16 = mybir.dt.bfloat16
NPBF = ml_dtypes.bfloat16
AF = mybir.ActivationFunctionType
NEG_BIG = -1.0e9


def _build(debug=False):
    nc = bacc.Bacc("TRN2", target_bir_lowering=False, debug=False,
                   num_devices=N_CORES)
    xT = nc.dram_tensor("xT", [D, S], BF16, kind="ExternalInput").ap()
    wqT = nc.dram_tensor("wqT", [D, 256], BF16, kind="ExternalInput").ap()
    wkT = nc.dram_tensor("wkT", [D, 256], BF16, kind="ExternalInput").ap()
    wvT = nc.dram_tensor("wvT", [D, 256], BF16, kind="ExternalInput").ap()
    woT = nc.dram_tensor("woT", [256, D], BF16, kind="ExternalInput").ap()
    bq2 = nc.dram_tensor("bq2", [128, 2], F32, kind="ExternalInput").ap()
    tri = nc.dram_tensor("tri", [128, 128], BF16, kind="ExternalInput").ap()
    y = nc.dram_tensor("y", [S, D], BF16, kind="ExternalOutput").ap()
    dbg = {}
    if debug:
        for nm, shp in [("qT", [128, 2, S]), ("kT", [128, 2, S]),
                        ("vv", [128, 16, 260]), ("oT", [128, 2, S])]:
            dbg[nm] = nc.dram_tensor(nm, shp, BF16, kind="ExternalOutput").ap()

    NQC = 4          # q-chunks of 512
    QC = 512
    NKT = S // 128   # k tiles

    with tile.TileContext(nc) as tc, \
            nc.allow_low_precision(reason="bf16 attention kernel"):
        with (
            tc.tile_pool(name="persist", bufs=1) as persist,
            tc.tile_pool(name="kqv", bufs=2) as kqv,
        ):
            qT_sb = [kqv.tile([128, S], BF16, tag="qT", name=f"qT{p}") for p in range(2)]
            kT_sb = [kqv.tile([128, S], BF16, tag="kT", name=f"kT{p}") for p in range(2)]
            v_sb = [persist.tile([128, 4 * 65], BF16, tag=f"v{t}", name=f"v{t}")
                    for t in range(NKT)]
            outT_sb = [persist.tile([128, S], BF16, tag=f"oT{p}", name=f"oTs{p}")
                       for p in range(2)]
            wo_sb = [persist.tile([128, D], BF16, tag=f"wo{p}", name=f"wo{p}")
                     for p in range(2)]
            tri_sb = persist.tile([128, 128], BF16, tag="tri")
            bq_sb = persist.tile([128, 2], F32, tag="bq")

            with (
                tc.tile_pool(name="xw", bufs=1) as xw,
                tc.tile_pool(name="ep", bufs=7) as ep,
                tc.tile_pool(name="rp", bufs=6) as rp,
            ):
                xt = [xw.tile([128, S], BF16, tag=f"x{c}", name=f"xt{c}") for c in range(8)]
                wq_sb = [xw.tile([128, 256], BF16, tag=f"wq{c}", name=f"wqs{c}") for c in range(8)]
                wk_sb = [xw.tile([128, 256], BF16, tag=f"wk{c}", name=f"wks{c}") for c in range(8)]
                wv_sb = [xw.tile([128, 256], BF16, tag=f"wv{c}", name=f"wvs{c}") for c in range(8)]
                # xt on the sync HWDGE queue; wq on the scalar HWDGE queue;
                # wk/wv interleaved on gpsimd so chunk c lands just before
                # the xt chunk that gates its preamble matmul.
                for c in range(8):
                    nc.sync.dma_start(out=xt[c], in_=xT[c * 128:(c + 1) * 128, :])
                for c in range(8):
                    nc.scalar.dma_start(out=wq_sb[c], in_=wqT[c * 128:(c + 1) * 128, :])
                for c in range(8):
                    nc.gpsimd.dma_start(out=wk_sb[c], in_=wkT[c * 128:(c + 1) * 128, :])
                    nc.gpsimd.dma_start(out=wv_sb[c], in_=wvT[c * 128:(c + 1) * 128, :])
                nc.gpsimd.dma_start(out=bq_sb, in_=bq2)
                nc.gpsimd.dma_start(out=tri_sb, in_=tri)
                for p in range(2):
                    nc.gpsimd.dma_start(out=wo_sb[p], in_=woT[p * 128:(p + 1) * 128, :])

                # trace-order schedule validator: a read before its write in
                # trace order silently reads garbage, so assert every block's
                # needs were emitted earlier.
                written = set()

                def qk_chain(p, j, which, pool):
                    written.add((which, p, j))
                    ps = pool.tile([128, QC], F32, tag="proj", name="ps")
                    w_sb = wq_sb if which == "q" else wk_sb
                    for c in range(8):
                        nc.tensor.matmul(
                            ps, w_sb[c][:, p * 128:(p + 1) * 128],
                            xt[c][:, j * QC:(j + 1) * QC],
                            start=(c == 0), stop=(c == 7))
                    if which == "q":
                        nc.vector.tensor_scalar_add(
                            qT_sb[p][:, j * QC:(j + 1) * QC], ps, bq_sb[:, p:p + 1])
                    else:
                        nc.vector.tensor_copy(kT_sb[p][:, j * QC:(j + 1) * QC], ps)

                def v_chain(t, pool):
                    written.add(("v", t))
                    ps_v = pool.tile([128, 256], F32, tag="proj", name="ps_v")
                    for c in range(8):
                        nc.tensor.matmul(
                            ps_v, xt[c][:, t * 128:(t + 1) * 128], wv_sb[c],
                            start=(c == 0), stop=(c == 7))
                    v_view = v_sb[t].rearrange("p (h w) -> p h w", w=65)
                    nc.vector.memset(v_view[:, :, 64:65], 1.0)
                    nc.vector.tensor_copy(
                        v_view[:, :, 0:64],
                        ps_v.rearrange("p (h w) -> p h w", w=64))

                def emit_norm(p_, q0_, o_ps_, tail=False):
                    written.add(("outT", p_, q0_ // QC))
                    # recip of the ones-row sum (copied to SBUF first —
                    # reciprocal_approx_fast from PSUM returns garbage),
                    # broadcast along partitions on GpSimd, fused multiply
                    # from PSUM into the bf16 outT tile. In the kernel tail
                    # (last block) the sums copy goes to the idle ScalarE and
                    # the multiply is split per q-tile so the output
                    # projection units start sooner.
                    if tail:
                        # 256-wide chunks: the first output-projection units
                        # unlock after one chunk-chain (~2us) instead of the
                        # full-width chain (~4us), keeping the PE-idle gap
                        # under the HAM re-throttle window.
                        for qq in range(2):
                            cs = slice(qq * 256, (qq + 1) * 256)
                            for s in range(2):
                                sums = rp.tile([1, 256], F32, tag="sums_t",
                                               name="sums_t")
                                nc.scalar.activation(
                                    sums, o_ps_[s][64:65, cs], AF.Copy)
                                recip = rp.tile([1, 256], F32, tag="recip_t",
                                                name="recip_t")
                                nc.vector.reciprocal_approx_fast(
                                    out=recip, in_=sums)
                                bc = rp.tile([64, 256], F32, tag="bc_t",
                                             name="bc_t")
                                nc.gpsimd.partition_broadcast(bc, recip)
                                nc.vector.tensor_mul(
                                    outT_sb[p_][s * 64:(s + 1) * 64,
                                                q0_ + qq * 256:q0_ + (qq + 1) * 256],
                                    o_ps_[s][0:64, cs], bc)
                        return
                    for s in range(2):
                        sums = rp.tile([1, QC], F32, tag="sums", name="sums")
                        nc.vector.tensor_copy(sums, o_ps_[s][64:65, :])
                        recip = rp.tile([1, QC], F32, tag="recip", name="recip")
                        nc.vector.reciprocal_approx_fast(out=recip, in_=sums)
                        bc = rp.tile([64, QC], F32, tag="bc", name="bc")
                        nc.gpsimd.partition_broadcast(bc, recip)
                        nc.vector.tensor_mul(
                            outT_sb[p_][s * 64:(s + 1) * 64, q0_:q0_ + QC],
                            o_ps_[s][0:64, :], bc)

                def emit_pv(p, o_ps, nkt, ent):
                    _kt, _e, _lo = ent
                    for s in range(2):
                        hb = 2 * p + s
                        nc.tensor.matmul(
                            o_ps[s][:, _lo:QC],
                            v_sb[_kt][:, hb * 65:(hb + 1) * 65],
                            _e[:, s * QC + _lo:(s + 1) * QC],
                            start=(_kt == 0), stop=(_kt == nkt - 1),
                            skip_group_check=True)

                def emit_block(p, qc, pops, tail=False):
                    assert ("q", p, qc) in written, (p, qc, "q")
                    for j in range(qc + 1):
                        assert ("k", p, j) in written, (p, qc, "k", j)
                    for t in range(4 * qc + 4):
                        assert ("v", t) in written, (p, qc, "v", t)
                    q0 = qc * QC
                    nkt = 4 * qc + 4
                    o_ps = [opp.tile([65, QC], F32, tag=f"o{s}", name=f"ops{s}")
                            for s in range(2)]
                    pend = []
                    for kt in range(nkt):
                        o = kt * 128 - q0
                        diag = o >= 0
                        lo = o if diag else 0
                        s_ab = sqp.tile([128, 2 * QC], F32, tag="sq", name="s_ab")
                        for s in range(2):
                            half = s * QC
                            nc.tensor.matmul(
                                s_ab[:, half + lo:half + QC],
                                kT_sb[p][s * 64:(s + 1) * 64,
                                         kt * 128:(kt + 1) * 128],
                                qT_sb[p][s * 64:(s + 1) * 64,
                                         q0 + lo:q0 + QC],
                                start=True, stop=True,
                                tile_position=(s * 64, 0),
                                skip_group_check=True)
                        e_ab = ep.tile([128, 2 * QC], BF16, tag="e", name="e_ab")
                        if diag:
                            # one ACTIVATE spanning both heads; cols
                            # [QC:QC+lo] hold exp(stale psum) and are
                            # never read downstream.
                            nc.scalar.activation(
                                e_ab[:, lo:2 * QC], s_ab[:, lo:2 * QC],
                                AF.Exp, scale=0.125)
                            for s in range(2):
                                nc.vector.tensor_mul(
                                    e_ab[:, s * QC + o:s * QC + o + 128],
                                    e_ab[:, s * QC + o:s * QC + o + 128],
                                    tri_sb)
                        else:
                            nc.scalar.activation(e_ab, s_ab, AF.Exp, scale=0.125)
                        # PV lags scores by 3 kt: exp(kt) gets ~3 PE rounds of
                        # slack, the first PV of a block lands after the prior
                        # block's norm has released the o_ps slots, and the
                        # end-of-block flush leaves a PE burst at the boundary.
                        if len(pend) == 4:
                            emit_pv(p, o_ps, nkt, pend.pop(0))
                        pops(kt, nkt)
                        pend.append((kt, e_ab, lo))
                    while pend:
                        emit_pv(p, o_ps, nkt, pend.pop(0))
                    emit_norm(p, q0, o_ps, tail=tail)

                # ---- preamble: pair-0 j0 q/k + v0-3, interleaved c-major so
                # every arriving x chunk unlocks 6 matmuls. (Extending this
                # with pair-1 j0 measured 200us vs 164.5 — do not retry.) ----
                with tc.tile_pool(name="ppsA", bufs=6, space="PSUM") as ppsA:
                    ps_q0 = ppsA.tile([128, QC], F32, tag="projA", name="ps_q0")
                    ps_k0 = ppsA.tile([128, QC], F32, tag="projA", name="ps_k0")
                    ps_vh = [ppsA.tile([128, 256], F32, tag="projA",
                                       name=f"ps_vh{t}") for t in range(4)]
                    for c in range(8):
                        nc.tensor.matmul(
                            ps_q0, wq_sb[c][:, 0:128], xt[c][:, 0:QC],
                            start=(c == 0), stop=(c == 7))
                        nc.tensor.matmul(
                            ps_k0, wk_sb[c][:, 0:128], xt[c][:, 0:QC],
                            start=(c == 0), stop=(c == 7))
                        for t in range(4):
                            nc.tensor.matmul(
                                ps_vh[t], xt[c][:, t * 128:(t + 1) * 128],
                                wv_sb[c], start=(c == 0), stop=(c == 7))
                    written.update({("q", 0, 0), ("k", 0, 0),
                                    ("v", 0), ("v", 1), ("v", 2), ("v", 3)})
                    nc.vector.tensor_scalar_add(
                        qT_sb[0][:, 0:QC], ps_q0, bq_sb[:, 0:1])
                    nc.vector.tensor_copy(kT_sb[0][:, 0:QC], ps_k0)
                    for t in range(4):
                        v_view = v_sb[t].rearrange("p (h w) -> p h w", w=65)
                        nc.vector.memset(v_view[:, :, 64:65], 1.0)
                        nc.vector.tensor_copy(
                            v_view[:, :, 0:64],
                            ps_vh[t].rearrange("p (h w) -> p h w", w=64))
                # ---- interleaved pair-0 / pair-1 attention blocks, with
                # projection chains and output-projection units as fillers ----
                with (
                    tc.tile_pool(name="sq", bufs=2, space="PSUM") as sqp,
                    tc.tile_pool(name="ops", bufs=1, space="PSUM") as opp,
                    tc.tile_pool(name="aux", bufs=2, space="PSUM") as aux,
                    tc.tile_pool(name="fsb", bufs=4) as fsb,
                ):
                    f_hold = {}

                    def c_unit(qt, oc):
                        for p_ in range(2):
                            assert ("outT", p_, qt // 4) in written, (qt, oc, p_)
                        f_ps = aux.tile([128, 512], F32, tag="proj", name="f_ps")
                        for p in range(2):
                            nc.tensor.matmul(
                                f_ps, outT_sb[p][:, qt * 128:(qt + 1) * 128],
                                wo_sb[p][:, oc * 512:(oc + 1) * 512],
                                start=(p == 0), stop=(p == 1))
                        if oc == 0:
                            f_sb = fsb.tile([128, 1024], BF16, tag="f",
                                            name="f_sb")
                            f_hold[qt] = f_sb
                        else:
                            f_sb = f_hold.pop(qt)
                        nc.vector.tensor_copy(
                            f_sb[:, oc * 512:(oc + 1) * 512], f_ps)
                        if oc == 1:
                            nc.sync.dma_start(
                                out=y[qt * 128:(qt + 1) * 128, :], in_=f_sb)

                    def qk(p, j, w):
                        return lambda: qk_chain(p, j, w, aux)

                    def vch(t):
                        return lambda: v_chain(t, aux)

                    # Block order mixes ScalarE-heavy attention with PE-heavy
                    # projections and ends on the smallest block (1,1) to
                    # shrink the tail.
                    blocks = [(0, 0), (0, 1), (1, 0), (0, 2),
                              (1, 1), (0, 3), (1, 2), (1, 3)]
                    # deadline table: block (p,qc) needs its pair's k chunks
                    # j<=qc and q chunk j=qc written in an EARLIER block (the
                    # preamble covers pair-0 j0 and v0-3).
                    # fillers pop as late as their deadline allows so the
                    # ScalarE-paced late blocks keep the PE warm.
                    queues = {i: [] for i in range(8)}
                    queues[0] = [qk(0, 1, "q"), qk(0, 1, "k"),
                                 vch(4), vch(5), vch(6), vch(7)]
                    queues[1] = [qk(1, 0, "k"), qk(1, 0, "q"),
                                 qk(0, 2, "q"), qk(0, 2, "k"),
                                 vch(8), vch(9), vch(10), vch(11)]
                    queues[2] = [qk(1, 1, "k"), qk(1, 1, "q"), vch(12), vch(13)]
                    queues[4] = [qk(0, 3, "q"), qk(0, 3, "k"), vch(14), vch(15),
                                 qk(1, 2, "k"), qk(1, 2, "q")]
                    queues[5] = [qk(1, 3, "k"), qk(1, 3, "q")]
                    # c_unit routing: units for qc_j may only run after BOTH
                    # (0,j) and (1,j) blocks have produced outT for qt range.
                    unit_route = {2: [(3, 8)], 4: [(5, 4), (6, 4)], 6: [(7, 8)]}

                    # queues 3 and 7 hold only c_units produced by the
                    # immediately-preceding block; popping them at kt=0 would
                    # HOL-block the PE FIFO on that block's ~3us norm chain,
                    # so delay their first pop a few kt.
                    pop_delay = {3: 3, 7: 3}

                    def make_pops(bi):
                        # at most one filler per kt slot; leftovers drain at
                        # the block boundary, giving the PE guaranteed work
                        # across the norm chain so HAM stays warm.
                        def pops(kt, nkt):
                            q = queues[bi]
                            if q and kt >= pop_delay.get(bi, 0):
                                q.pop(0)()
                        return pops

                    for bi, (p, qc) in enumerate(blocks):
                        emit_block(p, qc, make_pops(bi),
                                   tail=(bi == len(blocks) - 1))
                        while queues[bi]:
                            queues[bi].pop(0)()
                        if p == 1:
                            units = [lambda qt=qt, oc=oc: c_unit(qt, oc)
                                     for qt in range(qc * 4, (qc + 1) * 4)
                                     for oc in range(2)]
                            for tgt, n in unit_route.get(bi, []):
                                queues[tgt].extend(units[:n])
                                units = units[n:]
                            for u in units:
                                u()

            if debug:
                for p in range(2):
                    nc.sync.dma_start(out=dbg["qT"][:, p, :], in_=qT_sb[p])
                    nc.sync.dma_start(out=dbg["kT"][:, p, :], in_=kT_sb[p])
                    nc.sync.dma_start(out=dbg["oT"][:, p, :], in_=outT_sb[p])
                for t in range(NKT):
                    nc.sync.dma_start(out=dbg["vv"][:, t, :], in_=v_sb[t])

    nc.compile()
    return nc


_cached = {}


def _get_nc(debug=False):
    key = bool(debug)
    if key not in _cached:
        _cached[key] = _build(debug)
    return _cached[key]


def _prep_inputs(x, w_q, b_q, w_k, w_v):
    tri = np.triu(np.ones((128, 128), np.float32)).astype(NPBF)
    wqT_f = np.ascontiguousarray(w_q.T).astype(NPBF)
    wkT_f = np.ascontiguousarray(w_k.T).astype(NPBF)
    wvT_f = np.ascontiguousarray(w_v.T).astype(NPBF)
    in_maps = []
    for core in range(N_CORES):
        b, hg = divmod(core, 4)
        cs = slice(hg * 256, (hg + 1) * 256)
        in_maps.append({
            "xT": np.ascontiguousarray(x[b].T).astype(NPBF),
            "wqT": np.ascontiguousarray(wqT_f[:, cs]),
            "wkT": np.ascontiguousarray(wkT_f[:, cs]),
            "wvT": np.ascontiguousarray(wvT_f[:, cs]),
            "bq2": np.ascontiguousarray(
                b_q[hg * 256:(hg + 1) * 256].reshape(2, 128).T.astype(np.float32)),
            "tri": tri,
        })
    return in_maps


def _numpy_reference(x, attention_mask, w_q, b_q, w_k, b_k, w_v, b_v, w_o, b_o):
    x = x.astype(np.float64)
    q = (x @ w_q.T + b_q).reshape(B, S, H, DK).transpose(0, 2, 1, 3)
    k = (x @ w_k.T + b_k).reshape(B, S, H, DK).transpose(0, 2, 1, 3)
    v = (x @ w_v.T + b_v).reshape(B, S, H, DK).transpose(0, 2, 1, 3)
    scores = np.einsum("bhqd,bhkd->bhqk", q, k) / np.sqrt(DK)
    causal = np.tril(np.ones((S, S), bool))
    mask = causal[None, None] & (attention_mask[:, None, None, :] != 0)
    scores = np.where(mask, scores, -np.inf)
    scores -= scores.max(-1, keepdims=True)
    e = np.exp(scores)
    attn = e / e.sum(-1, keepdims=True)
    out = np.einsum("bhqk,bhkd->bhqd", attn, v)
    out = out.transpose(0, 2, 1, 3).reshape(B, S, D)
    return (out @ w_o.T + b_o).astype(np.float32)


def kernel(x, attention_mask, w_q, b_q, w_k, b_k, w_v, b_v, w_o, b_o,
           _debug=False, _trace=False):
    x = np.asarray(x, np.float32)
    attention_mask = np.asarray(attention_mask)
    if not np.all(attention_mask != 0):
        return _numpy_reference(np.asarray(x), np.asarray(attention_mask),
                                *[np.asarray(a) for a in
                                  (w_q, b_q, w_k, b_k, w_v, b_v, w_o, b_o)])
    w_q, w_k, w_v, w_o = [np.asarray(w, np.float32) for w in (w_q, w_k, w_v, w_o)]
    b_q, b_k, b_v, b_o = [np.asarray(b, np.float32) for b in (b_q, b_k, b_v, b_o)]

    nc = _get_nc(_debug)
    in_maps = _prep_inputs(x, w_q, b_q, w_k, w_v)
    woT_f = np.ascontiguousarray(w_o.T).astype(NPBF)
    for core in range(N_CORES):
        hg = core % 4
        in_maps[core]["woT"] = np.ascontiguousarray(
            woT_f[hg * 256:(hg + 1) * 256, :])

    res = run_bass_kernel_spmd(nc, in_maps, list(range(N_CORES)), trace=_trace)
    const_row = (b_v @ w_o.T + b_o).astype(np.float32)
    y = np.zeros((B, S, D), np.float32)
    for core in range(N_CORES):
        b = core // 4
        y[b] += res.results[core]["y"].astype(np.float32)
    y += const_row
    if _debug or _trace:
        return y, res
    return y

